# revision 2
# baseline (speedup 1.0000x reference)
"""Trainium2 Bass kernel for the ChebyshevBasis problem.

Computes, for x:[8192,512], coeffs:[512,512,16], base_weight:[512,512]:
    t = tanh(x); basis_n = T_n(t) (Chebyshev); out = einsum('bfn,fon->bo') + x@base_weight
Returns (out:[8192,512] f32, kl=zeros(1)).

Strategy (8 NeuronCores, data-parallel over batch):
  - Each core handles 1024 batch rows; weights replicated.
  - The contraction is one big matmul [1024, K=8192+512] @ [K, 512] done as
    64 K-tiles of 128 on the tensor engine in float32r (full-rate fp32).
  - Basis terms are built in transposed layout [feature, batch] so the
    contraction dim lands on SBUF partitions:
      * T_1 = tanh(x^T) on ScalarE
      * even T_2k = 2*T_k^2 - 1 (ScalarE Square + fused VectorE tensor_scalar)
      * odd  T_n  = 2*t*T_{n-1} - T_{n-2} (VectorE tensor_tensor + scalar_tensor_tensor)
      * T_0 == 1 is folded into a per-output bias = sum_f coeffs[f,o,0] (host
        weight repack), added during the PSUM->SBUF copy.
  - x^T obtained with PE-transposes of 128x128 blocks.
"""

import numpy as np

B, F, O, DEG = 8192, 512, 512, 15
N_CORES = 8
BSH = B // N_CORES          # 1024 batch rows per core
CB = 256                    # batch chunk (2 blocks of 128)
NBLK = CB // 128            # 2
NCHUNK = BSH // CB          # 4
NT = DEG                    # 15 stored terms, n = 1..15
KT = NT * 4 + 4             # 64 k-tiles of 128 (15 terms * 4 fblocks + residual)

_CACHE = {}


def _build_bass():
    import concourse.bacc as bacc
    import concourse.mybir as mybir
    from concourse import masks
    from concourse.tile import TileContext

    AF = mybir.ActivationFunctionType
    OP = mybir.AluOpType
    F32 = mybir.dt.float32
    F32R = mybir.dt.float32r

    nc = bacc.Bacc("TRN2", target_bir_lowering=False, debug=False,
                   num_devices=N_CORES)
    xs_ext = nc.declare_dram_parameter("xs", [BSH, F], F32, isOutput=False)
    w_ext = nc.declare_dram_parameter("w", [KT, 128, O], F32, isOutput=False)
    bias_ext = nc.declare_dram_parameter("bias", [128, O], F32, isOutput=False)
    out_ext = nc.declare_dram_parameter("out", [BSH, O], F32, isOutput=True)

    with TileContext(nc) as tc:
        from contextlib import ExitStack
        es = ExitStack()
        with es:
            wpool = es.enter_context(tc.tile_pool(name="w", bufs=1))
            fixed = es.enter_context(tc.tile_pool(name="fixed", bufs=1))
            stage = es.enter_context(tc.tile_pool(name="stage", bufs=2))
            xtp = es.enter_context(tc.tile_pool(name="xt", bufs=2))
            xtrp = es.enter_context(tc.tile_pool(name="xtr", bufs=2))
            termp = es.enter_context(tc.tile_pool(name="term", bufs=8))
            sqp = es.enter_context(tc.tile_pool(name="sq", bufs=2))
            ptp = es.enter_context(tc.tile_pool(name="pt", bufs=2))
            outp = es.enter_context(tc.tile_pool(name="out", bufs=2))
            ps_t = es.enter_context(tc.tile_pool(name="ps_t", bufs=2, space="PSUM"))
            ps_acc = es.enter_context(tc.tile_pool(name="ps_acc", bufs=4, space="PSUM"))

            ident = fixed.tile([128, 128], F32, tag="ident")
            masks.make_identity(nc, ident[:])
            bias_sb = fixed.tile([128, O], F32, tag="bias")
            nc.sync.dma_start(out=bias_sb[:], in_=bias_ext[:])

            # resident weights: 8 groups of 8 k-tiles, [128, 8*512] each
            wt = []
            for g in range(8):
                wg = wpool.tile([128, 8 * O], F32R, tag=f"w{g}")
                for i in range(8):
                    kt = g * 8 + i
                    nc.sync.dma_start(out=wg[:, i * O:(i + 1) * O],
                                      in_=w_ext[kt].bitcast(F32R))
                wt.append(wg)

            def wslice(kt):
                g, i = divmod(kt, 8)
                return wt[g][:, i * O:(i + 1) * O]

            def prologue(c):
                """Load + transpose x for chunk c; compute T1. Returns (xT, xTr, T1)."""
                xT = xtp.tile([128, 4 * CB], F32, tag="xt")
                for j in range(NBLK):
                    bb = c * NBLK + j
                    st = stage.tile([128, F], F32, tag="xst")
                    nc.sync.dma_start(out=st[:], in_=xs_ext[bb * 128:(bb + 1) * 128, :])
                    for fs in range(4):
                        pst = ps_t.tile([128, 128], F32, tag="pst")
                        nc.tensor.transpose(pst[:], st[:, fs * 128:(fs + 1) * 128],
                                            ident[:])
                        nc.vector.tensor_copy(
                            xT[:, fs * CB + j * 128: fs * CB + j * 128 + 128], pst[:])
                xTr = xtrp.tile([128, 4 * CB], F32R, tag="xtr")
                nc.vector.tensor_copy(xTr[:], xT[:])
                t1 = termp.tile([128, 4 * CB], F32R, tag="term")
                nc.scalar.activation(t1[:], xT[:], AF.Tanh)
                return xT, xTr, t1

            nxt = prologue(0)
            for c in range(NCHUNK):
                _, xTr, t1 = nxt
                acc = [ps_acc.tile([128, O], F32, tag="acc", name=f"acc{c}_{j}")
                       for j in range(NBLK)]
                T = {1: t1}

                def emit_mms(n, tile):
                    ti = n - 1
                    for fb in range(4):
                        kt = ti * 4 + fb
                        for j in range(NBLK):
                            nc.tensor.matmul(
                                acc[j][:],
                                tile[:, fb * CB + j * 128: fb * CB + j * 128 + 128],
                                wslice(kt), start=(kt == 0), stop=False)

                emit_mms(1, T[1])
                for n in range(2, NT + 1):
                    tn = termp.tile([128, 4 * CB], F32R, tag="term")
                    if n % 2 == 0:
                        k = n // 2
                        sq = sqp.tile([128, 4 * CB], F32, tag="sq")
                        nc.scalar.activation(sq[:], T[k][:], AF.Square)
                        nc.vector.tensor_scalar(tn[:], sq[:], 2.0, 1.0,
                                                OP.mult, OP.subtract)
                    else:
                        p = ptp.tile([128, 4 * CB], F32, tag="pt")
                        nc.vector.tensor_tensor(p[:], T[1][:], T[n - 1][:], OP.mult)
                        nc.vector.scalar_tensor_tensor(tn[:], p[:], 2.0, T[n - 2][:],
                                                       OP.mult, OP.subtract)
                    T[n] = tn
                    emit_mms(n, tn)
                    if n == 2 and c + 1 < NCHUNK:
                        nxt = prologue(c + 1)

                # residual k-tiles from x^T
                for fb in range(4):
                    kt = NT * 4 + fb
                    for j in range(NBLK):
                        nc.tensor.matmul(
                            acc[j][:],
                            xTr[:, fb * CB + j * 128: fb * CB + j * 128 + 128],
                            wslice(kt), start=False, stop=(kt == KT - 1))

                for j in range(NBLK):
                    ob = outp.tile([128, O], F32, tag="ob")
                    nc.vector.tensor_tensor(ob[:], acc[j][:], bias_sb[:], OP.add)
                    bb = c * NBLK + j
                    nc.sync.dma_start(out=out_ext[bb * 128:(bb + 1) * 128, :],
                                      in_=ob[:])

    nc.compile()
    return nc


def _repack_weights(coeffs, base_weight):
    w = np.empty((KT, 128, O), dtype=np.float32)
    for n in range(1, NT + 1):
        for fb in range(4):
            w[(n - 1) * 4 + fb] = coeffs[fb * 128:(fb + 1) * 128, :, n]
    for fb in range(4):
        w[NT * 4 + fb] = base_weight[fb * 128:(fb + 1) * 128, :]
    bias = coeffs[:, :, 0].sum(axis=0, dtype=np.float64).astype(np.float32)
    bias_rep = np.ascontiguousarray(np.broadcast_to(bias[None, :], (128, O)))
    return w, bias_rep


def kernel(x, coeffs, base_weight, _run_kwargs=None):
    from concourse.bass_utils import run_bass_kernel_spmd

    x = np.asarray(x, dtype=np.float32)
    coeffs = np.asarray(coeffs, dtype=np.float32)
    base_weight = np.asarray(base_weight, dtype=np.float32)

    if "nc" not in _CACHE:
        _CACHE["nc"] = _build_bass()
    nc = _CACHE["nc"]

    w, bias_rep = _repack_weights(coeffs, base_weight)
    in_maps = [
        {"xs": np.ascontiguousarray(x[c * BSH:(c + 1) * BSH]),
         "w": w, "bias": bias_rep}
        for c in range(N_CORES)
    ]
    res = run_bass_kernel_spmd(nc, in_maps, list(range(N_CORES)),
                               **(_run_kwargs or {}))
    out = np.concatenate([res.results[c]["out"] for c in range(N_CORES)], axis=0)
    kl = np.zeros((1,), dtype=np.float32)
    if _run_kwargs:
        _CACHE["last_results"] = res
    return out, kl
